# revision 80
# baseline (speedup 1.0000x reference)
"""HardAttention Bass kernel for 8 TRN2 NeuronCores (v2).

reference math (B=32, T=4096, H=256):
  energy[b,t,h] = relu( sum_k cat(hidden,enc)[b,t,k] * attn_w[h,k] + attn_b[h] )
  scores[b,t]   = sum_h energy[b,t,h] * v[h]
  out           = softmax(scores, axis=t)[:, None, :]

Device strategy (data-parallel over B, 4 batches/core), on-chip data fp16:
  * split attn_w into W1 (hidden half) and W2 (encoder half); fold v into
    W2 (w2v) and into the per-batch bias qv = (hidden@W1.T + attn_b)*v
    (valid because v >= 0: relu(x)*v == relu(x*v))
  * per (chunk of 512 t, b): z[h,t] = w2v.T-tiles @ enc-tiles (fp16 matmuls,
    f32 PSUM, h-chunks in separate PSUM pools for WAR slack); r0 =
    relu(z0+q0) on ACT; ONE DVE scalar_tensor_tensor forms
    r01 = max(z1,-q1) + r0 -- exact up to a per-batch constant shift of the
    scores, which softmax cancels (relu(x+q) = max(x,-q) + q)
  * ONE [128,128] PSUM scores tile: per (b,chunk) four quarter-width
    indicator matmuls (sliding-window lhsT, col b*32+4c+q hot) add the
    h-reduction of r01[:, q*128:...] into row b*32+4c+q — same PE rows as
    one wide matmul, but the exp/scale/output tail shrinks 4x; all inds
    form one PSUM accumulation group, emitted LAG units behind their
    producers so the in-order PE never waits; the final unit skips the
    merge and reduces r0/r1 separately
  * tail: one ACT exp+accum (e in bf16); per-batch sums via one f32 PE
    matmul with a block-diagonal indicator (S_b replicated across each
    8-row group, written into a dead zpb PSUM buffer to avoid a WAR wait
    on the exp); DVE reciprocal + 4x-mode scale to fp16; single out DMA
  * a single dummy matmul at t~0 starts the PE p-state ramp clock, so the
    stream runs at full clock; the first enc DMA issues before the const
    DMAs and is split so each batch's tiles land just ahead of its compute
Host side: pack enc as [chunk, 128, (b,kc), 512] fp16; unscramble rows
(b*32+4c+q, cols t%128) of the [128,128] fp16 output back to [4, T] f32.
"""

from contextlib import ExitStack

import numpy as np

import concourse.tile as tile
from concourse import bacc, mybir
from concourse.bass_utils import run_bass_kernel_spmd

B, T, H = 32, 4096, 256
NCORES = 8
BC = B // NCORES            # 4 batches per core
KC = H // 128               # 2 k-chunks
HC = H // 128               # 2 h-chunks
CHUNK = 512
NCHUNK = T // CHUNK         # 8
NBLK = BC * KC              # 8 (b,kc) blocks per chunk
LAG = 4                     # deferred indicator matmuls (units)

F32 = mybir.dt.float32
FP16 = mybir.dt.float16
BF16 = mybir.dt.bfloat16

_CACHE = {}
LAST_RESULTS = None


def _build():
    if "nc" in _CACHE:
        return _CACHE["nc"]

    nc = bacc.Bacc(None, target_bir_lowering=False)
    enc_d = nc.dram_tensor("enc", [NCHUNK, 128, NBLK * CHUNK], FP16,
                           kind="ExternalInput")
    # cols [0:512) = w2v lhsT tiles (kc,hc); [512:767) = sliding indicator
    # (col 512+127 is all-ones)
    wc_d = nc.dram_tensor("wconst", [128, 512 + 255], FP16, kind="ExternalInput")
    qv_d = nc.dram_tensor("qv", [128, BC * HC + 128], F32, kind="ExternalInput")
    out_d = nc.dram_tensor("scores", [128, CHUNK // 4], FP16, kind="ExternalOutput")

    AF = mybir.ActivationFunctionType
    ALU = mybir.AluOpType

    with tile.TileContext(nc) as tc, ExitStack() as ctx:
        const = ctx.enter_context(tc.tile_pool(name="const", bufs=1))
        encp = ctx.enter_context(tc.tile_pool(name="encp", bufs=5))
        rp = ctx.enter_context(tc.tile_pool(name="rp", bufs=8))
        r01p = ctx.enter_context(tc.tile_pool(name="r01p", bufs=12))
        zpa = ctx.enter_context(tc.tile_pool(name="zpa", bufs=3, space="PSUM"))
        zpb = ctx.enter_context(tc.tile_pool(name="zpb", bufs=4, space="PSUM"))
        scp = ctx.enter_context(tc.tile_pool(name="scp", bufs=1, space="PSUM"))
        tailp = ctx.enter_context(tc.tile_pool(name="tail", bufs=1))

        # PE warmup fodder: memset'd fp16 tile, no DMA dependency
        warm_src = tailp.tile([128, 128], FP16, tag="warm_src")
        nc.vector.memset(warm_src[:], 0.0)
        # hoist the ACT table load to t~0 (it costs 1.3us wherever it runs)
        warm_act = tailp.tile([128, 1], FP16, tag="warm_act")
        nc.scalar.activation(warm_act[:], warm_src[:, 0:1], AF.Relu)

        wc_sb = const.tile([128, 512 + 255], FP16, tag="wconst")
        qv_sb = const.tile([128, BC * HC + 128], F32, tag="qv")

        def w2v_ap(kc, hc):
            off = (kc * HC + hc) * 128
            return wc_sb[:, off: off + 128]

        def ind_ap(j):
            off = 512 + 127 - j
            return wc_sb[:, off: off + 128]

        # enc chunk tiles: [128, 8*512] fp16; chunks 0-1 split so each
        # batch's tiles land just ahead of its compute (DMA latency hiding)
        enc_t = []
        for c in range(min(3, NCHUNK)):
            et = encp.tile([128, NBLK * CHUNK], FP16, tag="enc")
            nsplit = {0: 4, 1: 2}.get(c, 2)
            step = NBLK * CHUNK // nsplit
            for i in range(nsplit):
                nc.sync.dma_start(et[:, i * step:(i + 1) * step],
                                  enc_d[c][:, i * step:(i + 1) * step])
                if c == 0 and i == 0:
                    # consts issue after the critical first enc piece
                    nc.sync.dma_start(wc_sb[:], wc_d[:])
                elif c == 0 and i == 2:
                    nc.sync.dma_start(qv_sb[:], qv_d[:])
            enc_t.append(et)

        # One early dummy matmul starts the PE p-state ramp clock at t~0
        # (pe_busy_start is set by the first PE instruction), so the real
        # stream at ~3.5us runs entirely at full clock. It writes psc,
        # which the first indicator matmul resets via start=True.
        psc = scp.tile([128, CHUNK // 4], F32, tag="psc")
        nc.tensor.matmul(psc[:, :128], warm_src[:], warm_src[:],
                         start=True, stop=True)

        queue = []

        def flush(limit):
            # scores row = b*32 + 4c + q, cols = t' % 128
            while len(queue) > limit:
                j2, last, rhss = queue.pop(0)
                for q in range(4):
                    for i, r in enumerate(rhss):
                        nc.tensor.matmul(
                            psc[:], ind_ap(j2 + q),
                            r[:, q * 128:(q + 1) * 128],
                            start=(j2 == 0 and q == 0 and i == 0),
                            stop=(last and q == 3 and i == len(rhss) - 1),
                        )

        unit = 0
        for c in range(NCHUNK):
            if c + 3 < NCHUNK:
                et = encp.tile([128, NBLK * CHUNK], FP16, tag="enc")
                nc.sync.dma_start(et[:], enc_d[c + 3][:])
                enc_t.append(et)
            for b in range(BC):
                def enc_ap(kc):
                    off = (b * KC + kc) * CHUNK
                    return enc_t[c][:, off: off + CHUNK]

                za = zpa.tile([128, CHUNK], F32, tag="za")
                zb = zpb.tile([128, CHUNK], F32, tag="zb")
                zs = [za[:], zb[:]]
                # unit 0: kc-major so compute starts on the first DMA piece
                loop = ([(hc, kc) for kc in range(KC) for hc in range(HC)]
                        if unit == 0 else
                        [(hc, kc) for hc in range(HC) for kc in range(KC)])
                for hc, kc in loop:
                    nc.tensor.matmul(
                        zs[hc], w2v_ap(kc, hc), enc_ap(kc),
                        start=(kc == 0), stop=(kc == KC - 1),
                    )
                flush(LAG)
                r0 = rp.tile([128, CHUNK], FP16, tag="r0")
                nc.scalar.activation(
                    r0[:], zs[0], AF.Relu,
                    bias=qv_sb[:, b * HC: b * HC + 1],
                )
                last = b == BC - 1 and c == NCHUNK - 1
                if unit >= BC * NCHUNK - 1:
                    # final unit: two ind matmuls; r1' = max(z1,-q1) runs on
                    # DVE in parallel with r0 on ACT (softmax-shift exact)
                    r1 = rp.tile([128, CHUNK], FP16, tag="r1")
                    nc.vector.tensor_scalar(
                        r1[:], zs[1],
                        scalar1=qv_sb[:, b * HC + 1: b * HC + 2],
                        scalar2=0.0, op0=ALU.max, op1=ALU.add,
                    )
                    queue.append((b * 32 + 4 * c, last, [r0, r1]))
                else:
                    # r01 = max(z1, -q1) + r0; the missing +q1 sums to a
                    # per-batch constant over h, which softmax cancels
                    r01 = r01p.tile([128, CHUNK], FP16, tag="r01")
                    nc.vector.scalar_tensor_tensor(
                        r01[:], zs[1],
                        qv_sb[:, b * HC + 1: b * HC + 2], r0[:],
                        op0=ALU.max, op1=ALU.add,
                    )
                    queue.append((b * 32 + 4 * c, last, [r01]))
                unit += 1
        flush(0)

        # softmax tail: one exp+accum; per-batch sums via one f32 PE matmul
        # (block-diagonal indicator replicates S_b across each 8-row group);
        # srep reuses psc col 0 (psc is dead after the exp reads it)
        e32 = tailp.tile([128, CHUNK // 4], BF16, tag="e32")
        acc = tailp.tile([128, 1], F32, tag="acc")
        nc.scalar.activation(e32[:], psc[:], AF.Exp, accum_out=acc[:])
        srep = zpb.tile([128, 1], F32, tag="zb")
        nc.tensor.matmul(srep[:], qv_sb[:, BC * HC:], acc[:],
                         start=True, stop=True)
        rec = tailp.tile([128, 1], F32, tag="rec")
        nc.vector.reciprocal(rec[:], srep[:])
        o16 = tailp.tile([128, CHUNK // 4], FP16, tag="o16")
        nc.vector.tensor_scalar_mul(o16[:], e32[:], rec[:, 0:1])
        nc.sync.dma_start(out_d[:], o16[:])

    nc.compile()
    _CACHE["nc"] = nc
    return nc


def _prep_inputs(hidden, encoder_outputs, attn_w, attn_b, v):
    w1 = attn_w[:, :H]
    w2 = attn_w[:, H:]
    qv_full = (((hidden @ w1.T) + attn_b) * v).astype(np.float32)   # [B, H]
    w2v_T = np.ascontiguousarray((w2 * v[:, None]).T)               # [k, h]

    wconst = np.zeros((128, 512 + 255), dtype=np.float16)
    for kc in range(KC):
        for hc in range(HC):
            off = (kc * HC + hc) * 128
            wconst[:, off: off + 128] = w2v_T[
                kc * 128:(kc + 1) * 128, hc * 128:(hc + 1) * 128
            ]
    wconst[:, 512 + 127] = 1.0

    in_maps = []
    for core in range(NCORES):
        bs = core * BC
        # enc[b, k, t] -> [chunk, 128(k%), (b,kc), 512]
        enc_c = encoder_outputs[:, bs: bs + BC, :].transpose(1, 2, 0)  # [BC,256,T]
        enc_c = enc_c.reshape(BC, KC, 128, NCHUNK, CHUNK)
        enc_c = np.ascontiguousarray(
            enc_c.transpose(3, 2, 0, 1, 4), dtype=np.float16
        ).reshape(NCHUNK, 128, NBLK * CHUNK)
        qv_c = np.zeros((128, BC * HC + 128), dtype=np.float32)
        qv_c[:, :BC * HC] = qv_full[bs: bs + BC].reshape(
            BC, HC, 128).transpose(2, 0, 1).reshape(128, BC * HC)
        qv_c[:, 1:BC * HC:2] *= -1.0  # hc1 columns hold -qv for the STT max
        for b in range(BC):
            qv_c[b * 32:(b + 1) * 32,
                 BC * HC + b * 32: BC * HC + (b + 1) * 32] = 1.0
        in_maps.append({"enc": enc_c, "wconst": wconst, "qv": qv_c})
    return in_maps


def kernel(hidden, encoder_outputs, attn_w, attn_b, v):
    global LAST_RESULTS
    nc = _build()
    in_maps = _prep_inputs(
        np.asarray(hidden, dtype=np.float32),
        np.asarray(encoder_outputs, dtype=np.float32),
        np.asarray(attn_w, dtype=np.float32),
        np.asarray(attn_b, dtype=np.float32),
        np.asarray(v, dtype=np.float32),
    )
    res = run_bass_kernel_spmd(nc, in_maps, list(range(NCORES)))
    LAST_RESULTS = res
    out = np.empty((B, 1, T), dtype=np.float32)
    for core in range(NCORES):
        sc = res.results[core]["scores"].astype(np.float32).reshape(
            BC, 32, CHUNK // 4)
        out[core * BC:(core + 1) * BC, 0, :] = sc.reshape(
            BC, NCHUNK * CHUNK)
    return out
